# revision 1
# baseline (speedup 1.0000x reference)
"""GCN decoder (nn_Decoder_87651692576924) on 8 Trainium2 NeuronCores.

Sharding (graph/data parallel per the hint):
  - 50000 graph nodes sharded contiguously across 8 cores (6250 each, padded
    to 6272 = 49*128); fc/conv weights replicated.
  - fc2 -> fc1 have no activation between them, so they fuse on the host into
    one affine map W_eff [32,16] (+ ones-row for the bias). The MLP input is
    host-permuted so each [128,256] output group lands node-major: contiguous
    512B-per-partition table writes and per-partition deg^-1/2 scaling.
  - All node-feature tables are bf16 and pre-scaled by deg^-1/2. Each layer,
    ONE AllGather publishes every core's [6272, 256] shard into a shared
    full table [8*6272, 256]; gather indices are split by source owner
    group (cores 0-3 vs 4-7, each indexing a half-table view) so they fit
    dma_gather's int16 index requirement.
  - Each core owns the edges whose dst lands in its shard (plus self-loops),
    host-sorted by (dst block, src owner group) and padded to 128-edge
    chunks. Gathers pull 128-row chunks from the shared table in 4-block
    windows (fewer SWDGE descriptor-gen calls), bf16 rows = 512B each.
  - Aggregation is transposed: per chunk, matmul(lhsT=msg_half [128e,128c],
    rhs=onehot [128e,128d]) accumulates aggT [c, dst] directly in PSUM
    across all chunks of a block (open accumulation groups), so no per-chunk
    PSUM->SBUF copies and no PE transposes. The conv matmul consumes aggT as
    its stationary operand; bias, ELU, and both deg^-1/2 scalings fuse into
    a short per-block epilogue.

Host-side numpy does only integer graph preprocessing (degree counts, edge
sort/pad/remap, layout) and weight folding; all per-node FLOPs (MLP, rsqrt
norms, messages, convs, ELU) run on device.
"""

import math
import sys
import time

import numpy as np

if "/opt/trn_rl_repo" not in sys.path:
    sys.path.insert(0, "/opt/trn_rl_repo")

import concourse.bass as bass
import concourse.tile as tile
from concourse import bacc, mybir

FP = mybir.dt.float32
BF = mybir.dt.bfloat16
AF = mybir.ActivationFunctionType
OP = mybir.AluOpType

P = 128

# ---------------- hardcoded problem configuration ----------------
N_GRAPHS = 50000
N_EDGES = 800000
NCORES = 8
INPUT_DIM = 16
IN_FEAT = 32
FFN = 128
HIDDEN = 16
C = INPUT_DIM * HIDDEN          # 256

SHARD = N_GRAPHS // NCORES      # 6250
NBLK = math.ceil(SHARD / P)     # 49
SHARD_PAD = NBLK * P            # 6272
NBLK_A = (NBLK + 1) // 2        # 25
NBLK_B = NBLK - NBLK_A          # 24
ROWS_A = NBLK_A * P             # 3200
ROWS_B = NBLK_B * P             # 3072
XROWS = SHARD_PAD * INPUT_DIM   # 100352

WINDOW = 4                      # dst blocks per dma_gather call
MLP_SLAB = 8                    # MLP groups per input DMA slab
OPEN_ACC = True                 # accumulate chunks in PSUM (open matmul groups)
DIRECT_TABLES = False           # publish shards by DMA + barrier (else AllGather)


# ---------------- host-side integer preprocessing ----------------
def _preprocess(edge_index):
    src = np.asarray(edge_index[0], dtype=np.int64)
    dst = np.asarray(edge_index[1], dtype=np.int64)
    loops = np.arange(N_GRAPHS, dtype=np.int64)
    s = np.concatenate([src, loops])
    d = np.concatenate([dst, loops])

    deg = np.bincount(d, minlength=N_GRAPHS).astype(np.float32)

    owner = d // SHARD
    dst_local = d - owner * SHARD
    blk = dst_local // P
    dst_in_blk = dst_local - blk * P

    s_owner = s // SHARD
    s_pos = s - s_owner * SHARD
    # split by source owner group (0-3 vs 4-7): both halves index into
    # views of ONE full AllGathered table and stay within int16
    in_a = s_owner < NCORES // 2
    row_half = np.where(in_a, s_owner * SHARD_PAD + s_pos,
                        (s_owner - NCORES // 2) * SHARD_PAD + s_pos
                        ).astype(np.int64)

    key = ((owner * NBLK + blk) * 2 + (~in_a).astype(np.int64))
    order = np.argsort(key, kind="stable")
    row_s = row_half[order]
    dib_s = dst_in_blk[order]

    cnt = np.bincount(key[order], minlength=NCORES * NBLK * 2)
    cntr = cnt.reshape(NCORES, NBLK, 2)
    k_req = np.maximum(1, -(-cntr // P))
    K = k_req.max(axis=0)
    kA = [int(v) for v in K[:, 0]]
    kB = [int(v) for v in K[:, 1]]

    starts = np.zeros(NCORES * NBLK * 2 + 1, dtype=np.int64)
    np.cumsum(cnt, out=starts[1:])

    per_core = []
    for r in range(NCORES):
        idx_half = {0: [], 1: []}
        sel_cols = []
        for b in range(NBLK):
            for h, kh in ((0, kA[b]), (1, kB[b])):
                gi = (r * NBLK + b) * 2 + h
                e0, e1 = starts[gi], starts[gi + 1]
                pad = kh * P - (e1 - e0)
                rows = np.concatenate(
                    [row_s[e0:e1], np.zeros(pad, dtype=np.int64)])
                sel = np.concatenate(
                    [dib_s[e0:e1], np.full(pad, 255, dtype=np.int64)])
                idx_half[h].append(rows)
                sel_cols.append(sel.reshape(kh, P).T)
        idxA = np.concatenate(idx_half[0]).astype(np.int16)
        idxB = np.concatenate(idx_half[1]).astype(np.int16)
        wrapA = np.tile(idxA.reshape(-1, 16).T, (8, 1))
        wrapB = np.tile(idxB.reshape(-1, 16).T, (8, 1))
        dst_sel = np.concatenate(sel_cols, axis=1)
        per_core.append(dict(idxA=wrapA, idxB=wrapB, dst_sel=dst_sel))
    return deg, per_core, dict(kA=kA, kB=kB)


def _to_bf16(a):
    """Round-to-nearest-even fp32 -> bf16, stored as uint16 words."""
    u = np.asarray(a, dtype=np.float32).view(np.uint32)
    rounded = (u + 0x7FFF + ((u >> 16) & 1)) >> 16
    return rounded.astype(np.uint16)


def _build_core_inputs(inputs, deg, per_core):
    x = np.asarray(inputs["x"], dtype=np.float32)
    fc2_w = np.asarray(inputs["fc2_w"], dtype=np.float32)
    fc2_b = np.asarray(inputs["fc2_b"], dtype=np.float32)
    fc1_w = np.asarray(inputs["fc1_w"], dtype=np.float32)
    fc1_b = np.asarray(inputs["fc1_b"], dtype=np.float32)

    w_eff = fc2_w @ fc1_w                       # [32, 16]
    b_eff = fc2_b @ fc1_w + fc1_b               # [16]
    w_aug = np.concatenate([w_eff, b_eff.reshape(1, -1)], axis=0)  # [33,16]

    iota = np.tile(np.arange(P, dtype=np.float32).reshape(1, P), (P, 8))

    shared = dict(w_aug=_to_bf16(w_aug), iota=_to_bf16(iota))
    for t in range(3):
        w = np.asarray(inputs[f"conv_w{t+1}"], dtype=np.float32)
        b = np.asarray(inputs[f"conv_b{t+1}"], dtype=np.float32)
        shared[f"w{t}"] = _to_bf16(
            np.concatenate([w[:P, :], w[P:, :]], axis=1))
        shared[f"bb{t}"] = _to_bf16(np.tile(b.reshape(1, -1), (P, 1)))

    in_maps = []
    for r in range(NCORES):
        m = dict(shared)
        xs = x[r * SHARD * INPUT_DIM:(r + 1) * SHARD * INPUT_DIM]
        xp = np.zeros((XROWS, IN_FEAT), dtype=np.float32)
        xp[:xs.shape[0]] = xs
        # permute rows so chunk c covers (node block c//16, row-in-node c%16):
        # column c*128+p  <->  x row 2048*(c//16) + 16*p + (c%16)
        xp = (xp.reshape(NBLK, P, INPUT_DIM, IN_FEAT)
              .transpose(0, 2, 1, 3)
              .reshape(XROWS, IN_FEAT))
        xt = np.concatenate(
            [xp.T, np.ones((1, XROWS), dtype=np.float32)], axis=0)  # [33, XROWS]
        m["xT"] = _to_bf16(xt)

        dg = np.ones(SHARD_PAD, dtype=np.float32)
        dg[:SHARD] = deg[r * SHARD:(r + 1) * SHARD]
        m["deg_blocks"] = dg.reshape(NBLK, P).T.copy()

        pc = per_core[r]
        m["idxA"], m["idxB"] = pc["idxA"], pc["idxB"]
        m["dst_sel"] = _to_bf16(pc["dst_sel"])
        in_maps.append(m)
    return in_maps


# ---------------- device program ----------------
def _build_program(meta, shapes):
    kA, kB = meta["kA"], meta["kB"]

    nc = bacc.Bacc("TRN2", target_bir_lowering=False, debug=False,
                   enable_asserts=True, num_devices=NCORES)

    np_dtype_map = {np.dtype(np.uint16): BF,
                    np.dtype(np.float32): FP,
                    np.dtype(np.int16): mybir.dt.int16}
    inp = {}
    for name, (shape, npdt) in shapes.items():
        inp[name] = nc.dram_tensor(
            name, list(shape), np_dtype_map[np.dtype(npdt)],
            kind="ExternalInput").ap()
    out_h = nc.dram_tensor("out_h", [SHARD_PAD, C], FP,
                           kind="ExternalOutput").ap()

    rg = [list(range(NCORES))]

    with tile.TileContext(nc) as tc:
        from contextlib import ExitStack
        estack = ExitStack()
        dram = estack.enter_context(
            tc.tile_pool(name="dram", bufs=1, space="DRAM"))
        tabF = [dram.tile([NCORES * SHARD_PAD, C], BF, addr_space="Shared",
                          name=f"tabF{t}") for t in range(3)]
        # local staging for this core's shard; one AllGather per layer
        # publishes it into the shared full table
        ccs = [dram.tile([SHARD_PAD, C], BF, name=f"cc{t}") for t in range(3)]
        bsrc = dram.tile([1, 16], FP, name="bsrc")
        bdst = [dram.tile([NCORES, 16], FP, addr_space="Shared",
                          name=f"bdst{t}") for t in range(3)]

        cpool = estack.enter_context(tc.tile_pool(name="const", bufs=1))

        def load_const(name, dtype):
            t = cpool.tile(list(shapes[name][0]), dtype, name=f"{name}_sb")
            nc.sync.dma_start(out=t[:], in_=inp[name][:])
            return t

        waug_sb = load_const("w_aug", BF)
        iota_sb = load_const("iota", BF)
        w_sb = [load_const(f"w{t}", BF) for t in range(3)]
        bb_sb = [load_const(f"bb{t}", BF) for t in range(3)]
        degb_sb = load_const("deg_blocks", FP)
        idxA_sb = load_const("idxA", mybir.dt.int16)
        idxB_sb = load_const("idxB", mybir.dt.int16)
        dsel_sb = load_const("dst_sel", BF)

        disqb = cpool.tile([P, NBLK], FP, name="disqb")
        nc.vector.reciprocal(disqb[:], degb_sb[:])
        nc.scalar.activation(disqb[:], disqb[:], AF.Sqrt)

        # barrier source (content irrelevant)
        binit = cpool.tile([1, 16], FP, name="binit")
        nc.vector.memset(binit[:], 0.0)
        nc.sync.dma_start(out=bsrc[:], in_=binit[:])

        # this core's rank: row base for its shard inside the shared tables
        pid = nc.partition_id()

        # ---------------- MLP: table 0 ----------------
        with tc.tile_pool(name="mlp_ps", bufs=2, space="PSUM") as pspool, \
             tc.tile_pool(name="mlp_x", bufs=2) as xpool, \
             tc.tile_pool(name="mlp_sb", bufs=3) as mlpsb:
            n_slab = math.ceil(NBLK / MLP_SLAB)
            for sl in range(n_slab):
                g0 = sl * MLP_SLAB
                g1 = min(NBLK, g0 + MLP_SLAB)
                ncols = (g1 - g0) * 16 * P
                xa = xpool.tile([IN_FEAT + 1, MLP_SLAB * 16 * P], BF,
                                name="xa", tag="xa")
                nc.sync.dma_start(
                    out=xa[:, :ncols],
                    in_=inp["xT"][:, g0 * 16 * P:g1 * 16 * P])
                for g in range(g0, g1):
                    off = (g - g0) * 16 * P
                    ps = pspool.tile([P, 512], FP, name="ps", tag="ps",
                                     space="PSUM")
                    for rr in range(16):
                        nc.tensor.matmul(
                            ps[:, rr * HIDDEN:(rr + 1) * HIDDEN],
                            lhsT=xa[:, off + rr * P:off + (rr + 1) * P],
                            rhs=waug_sb[:], start=True, stop=True)
                    pso = ps[:, 0:C]
                    m = mlpsb.tile([P, C], BF, name="m", tag="m")
                    nc.vector.tensor_scalar_min(m[:], pso, 0.0)
                    nc.scalar.activation(m[:], m[:], AF.Exp)
                    nc.vector.tensor_scalar(m[:], m[:], -1.0,
                                            disqb[:, g:g + 1],
                                            op0=OP.add, op1=OP.mult)
                    h2 = mlpsb.tile([P, C], BF, name="h2", tag="h2")
                    nc.vector.tensor_scalar(h2[:], pso, disqb[:, g:g + 1],
                                            None, op0=OP.mult)
                    ob = mlpsb.tile([P, C], BF, name="ob", tag="ob")
                    nc.vector.tensor_tensor(out=ob[:], in0=h2[:], in1=m[:],
                                            op=OP.max)
                    nc.sync.dma_start(out=ccs[0][g * P:(g + 1) * P, :],
                                      in_=ob[:])

        # ---------------- conv layers ----------------
        for t in range(3):
            if DIRECT_TABLES:
                # publish this core's shard into the shared table, barrier
                wF = nc.sync.dma_start(
                    out=tabF[t][bass.ds(pid * SHARD_PAD, SHARD_PAD), :],
                    in_=ccs[t][:])
                cc = nc.gpsimd.collective_compute(
                    "AllGather", OP.bypass, replica_groups=rg,
                    ins=[bsrc.opt()], outs=[bdst[t].opt()])
                bass._add_dep_helper(cc.ins, wF.ins, sync=True,
                                     reason="table publish before barrier")
            else:
                cc = None
                nc.gpsimd.collective_compute(
                    "AllGather", OP.bypass, replica_groups=rg,
                    ins=[ccs[t].opt()], outs=[tabF[t].opt()])

            colA_base = [8 * sum(kA[:b]) for b in range(NBLK + 1)]
            colB_base = [8 * sum(kB[:b]) for b in range(NBLK + 1)]
            kAw_max = max(sum(kA[b:b + WINDOW]) for b in range(0, NBLK, WINDOW))
            kBw_max = max(sum(kB[b:b + WINDOW]) for b in range(0, NBLK, WINDOW))

            kps_ctx = (tc.tile_pool(name=f"chunk_ps{t}", bufs=3, space="PSUM")
                       if not OPEN_ACC else None)
            with tc.tile_pool(name=f"agg_ps{t}", bufs=2, space="PSUM") as aps, \
                 tc.tile_pool(name=f"conv_ps{t}", bufs=2, space="PSUM") as cps, \
                 tc.tile_pool(name=f"gatA{t}", bufs=2) as gApool, \
                 tc.tile_pool(name=f"gatB{t}", bufs=2) as gBpool, \
                 tc.tile_pool(name=f"oh{t}", bufs=4) as ohpool, \
                 tc.tile_pool(name=f"csb{t}", bufs=3) as csb:
                kps = kps_ctx.__enter__() if kps_ctx is not None else None
                gatA = {}
                gatB = {}

                def issue_window(w0):
                    kaw = sum(kA[w0:w0 + WINDOW])
                    kbw = sum(kB[w0:w0 + WINDOW])
                    ga = gApool.tile([P, kAw_max * C], BF, name="ga", tag="ga")
                    g1 = nc.gpsimd.dma_gather(
                        out_ap=ga[:].rearrange("p (k e) -> p k e", e=C)[:, 0:kaw, :],
                        in_ap=tabF[t][:],
                        idxs_ap=idxA_sb[:, colA_base[w0]:colA_base[w0] + kaw * 8],
                        num_idxs=kaw * P, num_idxs_reg=kaw * P, elem_size=C,
                        single_packet=False)
                    gb = gBpool.tile([P, kBw_max * C], BF, name="gb", tag="gb")
                    g2 = nc.gpsimd.dma_gather(
                        out_ap=gb[:].rearrange("p (k e) -> p k e", e=C)[:, 0:kbw, :],
                        in_ap=tabF[t][(NCORES // 2) * SHARD_PAD:, :],
                        idxs_ap=idxB_sb[:, colB_base[w0]:colB_base[w0] + kbw * 8],
                        num_idxs=kbw * P, num_idxs_reg=kbw * P, elem_size=C,
                        single_packet=False)
                    if cc is not None:
                        for g in (g1, g2):
                            bass._add_dep_helper(g.ins, cc.ins, sync=True,
                                                 reason="gather after barrier")
                    gatA[w0] = ga
                    gatB[w0] = gb

                ck = 0
                for b in range(NBLK):
                    w0 = (b // WINDOW) * WINDOW
                    if b == w0:
                        issue_window(w0)
                    ga3 = gatA[w0][:].rearrange("p (k e) -> p k e", e=C)
                    gb3 = gatB[w0][:].rearrange("p (k e) -> p k e", e=C)
                    aoff = colA_base[b] // 8 - colA_base[w0] // 8
                    boff = colB_base[b] // 8 - colB_base[w0] // 8
                    kt = kA[b] + kB[b]

                    chunks = ([(ga3, aoff + j) for j in range(kA[b])]
                              + [(gb3, boff + j) for j in range(kB[b])])

                    # one-hot matrices for all chunks of the block, built
                    # 8 per DVE op (amortizes per-op overhead)
                    oh_blk = ohpool.tile([P, kt * P], BF, name="ohb",
                                         tag="ohb")
                    o3 = oh_blk[:].rearrange("p (n e) -> p n e", e=P)
                    c0 = 0
                    while c0 < kt:
                        nb = min(8, kt - c0)
                        sel3 = (dsel_sb[:, ck + c0:ck + c0 + nb]
                                .rearrange("p (n one) -> p n one", one=1)
                                .to_broadcast([P, nb, P]))
                        i3 = iota_sb[:, 0:nb * P].rearrange(
                            "p (n e) -> p n e", e=P)
                        nc.vector.tensor_tensor(
                            out=o3[:, c0:c0 + nb, :], in0=sel3, in1=i3,
                            op=OP.is_equal)
                        c0 += nb
                    ck += kt

                    if OPEN_ACC:
                        # one full PSUM bank per channel half: the start=True
                        # bank-wide has_written clear must not touch another
                        # in-flight accumulation
                        agg0 = aps.tile([P, 512], FP, name="agg0", tag="agg0",
                                        space="PSUM")
                        agg1 = aps.tile([P, 512], FP, name="agg1", tag="agg1",
                                        space="PSUM")
                        for k, (g3, kc) in enumerate(chunks):
                            oh = oh_blk[:, k * P:(k + 1) * P]
                            nc.tensor.matmul(
                                agg0[:, 0:P], lhsT=g3[:, kc, 0:P], rhs=oh,
                                start=(k == 0), stop=(k == kt - 1))
                            nc.tensor.matmul(
                                agg1[:, 0:P], lhsT=g3[:, kc, P:C], rhs=oh,
                                start=(k == 0), stop=(k == kt - 1))
                        aggT_sb = csb.tile([P, C], BF, name="aggT", tag="aggT")
                        nc.scalar.copy(aggT_sb[:, 0:P], agg0[:, 0:P])
                        nc.scalar.copy(aggT_sb[:, P:C], agg1[:, 0:P])
                    else:
                        acc = csb.tile([P, C], FP, name="acc", tag="acc")
                        for k, (g3, kc) in enumerate(chunks):
                            oh = oh_blk[:, k * P:(k + 1) * P]
                            cps_t = kps.tile([P, C], FP, name="ckps",
                                             tag="ckps", space="PSUM")
                            nc.tensor.matmul(cps_t[:, 0:P],
                                             lhsT=g3[:, kc, 0:P], rhs=oh,
                                             start=True, stop=True)
                            nc.tensor.matmul(cps_t[:, P:C],
                                             lhsT=g3[:, kc, P:C], rhs=oh,
                                             start=True, stop=True)
                            if k == 0:
                                nc.scalar.copy(acc[:], cps_t[:])
                            else:
                                nc.vector.tensor_tensor(
                                    out=acc[:], in0=acc[:], in1=cps_t[:],
                                    op=OP.add)
                        aggT_sb = csb.tile([P, C], BF, name="aggT", tag="aggT")
                        nc.vector.tensor_copy(aggT_sb[:], acc[:])

                    conv = cps.tile([P, 512], FP, name="conv", tag="conv",
                                    space="PSUM")
                    if OPEN_ACC:
                        nc.tensor.matmul(conv[:, 0:C],
                                         lhsT=aggT_sb[:, 0:P],
                                         rhs=w_sb[t][:, 0:C],
                                         start=True, stop=False)
                        nc.tensor.matmul(conv[:, 0:C],
                                         lhsT=aggT_sb[:, P:C],
                                         rhs=w_sb[t][:, C:2 * C],
                                         start=False, stop=True)
                    else:
                        nc.tensor.matmul(conv[:, 0:C],
                                         lhsT=aggT_sb[:, 0:P],
                                         rhs=w_sb[t][:, 0:C],
                                         start=True, stop=True)
                        nc.tensor.matmul(conv[:, C:2 * C],
                                         lhsT=aggT_sb[:, P:C],
                                         rhs=w_sb[t][:, C:2 * C],
                                         start=True, stop=True)

                    # epilogue: h = elu(disq*conv + b); next table = disq*h
                    # (final layer stays fp32 for output accuracy)
                    edt = BF if t < 2 else FP
                    h1 = csb.tile([P, C], edt, name="h1", tag="h1")
                    nc.scalar.activation(h1[:], conv[:, 0:C], AF.Identity,
                                         scale=disqb[:, b:b + 1])
                    if not OPEN_ACC:
                        h1b = csb.tile([P, C], edt, name="h1b", tag="h1b")
                        nc.scalar.activation(h1b[:], conv[:, C:2 * C],
                                             AF.Identity,
                                             scale=disqb[:, b:b + 1])
                        nc.vector.tensor_tensor(out=h1[:], in0=h1[:],
                                                in1=h1b[:], op=OP.add)
                    nc.vector.tensor_tensor(out=h1[:], in0=h1[:],
                                            in1=bb_sb[t][:], op=OP.add)
                    m = csb.tile([P, C], edt, name="em", tag="em")
                    nc.vector.tensor_scalar_min(m[:], h1[:], 0.0)
                    nc.scalar.activation(m[:], m[:], AF.Exp)
                    if t < 2:
                        nc.vector.tensor_scalar(m[:], m[:], -1.0,
                                                disqb[:, b:b + 1],
                                                op0=OP.add, op1=OP.mult)
                        ob = csb.tile([P, C], BF, name="ob2", tag="ob2")
                        nc.vector.tensor_scalar(ob[:], h1[:],
                                                disqb[:, b:b + 1], None,
                                                op0=OP.mult)
                        nc.vector.tensor_tensor(out=ob[:], in0=ob[:],
                                                in1=m[:], op=OP.max)
                        nc.sync.dma_start(
                            out=ccs[t + 1][b * P:(b + 1) * P, :], in_=ob[:])
                    else:
                        nc.vector.tensor_scalar_add(m[:], m[:], -1.0)
                        of = csb.tile([P, C], FP, name="of", tag="of")
                        nc.vector.tensor_tensor(out=of[:], in0=h1[:],
                                                in1=m[:], op=OP.max)
                        nc.sync.dma_start(out=out_h[b * P:(b + 1) * P, :],
                                          in_=of[:])
                if kps_ctx is not None:
                    kps_ctx.__exit__(None, None, None)

        estack.close()

    nc.compile()
    return nc


# ---------------- execution ----------------
_CACHE = {}


def _prepare(inputs):
    deg, per_core, meta = _preprocess(inputs["edge_index"])
    in_maps = _build_core_inputs(inputs, deg, per_core)
    shapes = {k: (v.shape, v.dtype) for k, v in in_maps[0].items()}
    nc = _build_program(meta, shapes)
    return nc, in_maps


def _assemble(results):
    out = np.empty((N_GRAPHS, C), dtype=np.float32)
    for r, res in enumerate(results):
        out[r * SHARD:(r + 1) * SHARD] = res["out_h"][:SHARD]
    return out


def kernel(**inputs):
    from concourse.bass_utils import run_bass_kernel_spmd
    nc, in_maps = _prepare(inputs)
    _CACHE["nc"], _CACHE["in_maps"] = nc, in_maps
    res = run_bass_kernel_spmd(nc, in_maps, core_ids=list(range(NCORES)))
    return _assemble(res.results)


def benchmark(repeats=5):
    """Re-execute the cached program with device-resident inputs; returns
    per-iteration wall times (s). Call after kernel()."""
    if "nc" not in _CACHE:
        return []
    import jax
    import numpy as _np
    from jax.sharding import Mesh, PartitionSpec
    from jax.experimental.shard_map import shard_map
    from concourse import bass2jax
    from concourse import mybir as mb

    nc, in_maps = _CACHE["nc"], _CACHE["in_maps"]
    bass2jax.install_neuronx_cc_hook()

    partition_name = (nc.partition_id_tensor.name
                      if nc.partition_id_tensor else None)
    in_names, out_names, out_avals, zero_outs = [], [], [], []
    for alloc in nc.m.functions[0].allocations:
        if not isinstance(alloc, mb.MemoryLocationSet):
            continue
        name = alloc.memorylocations[0].name
        if alloc.kind == "ExternalInput":
            if name != partition_name:
                in_names.append(name)
        elif alloc.kind == "ExternalOutput":
            out_names.append(name)
            shape = tuple(alloc.tensor_shape)
            dtype = mb.dt.np(alloc.dtype)
            out_avals.append(jax.core.ShapedArray(shape, dtype))
            zero_outs.append(_np.zeros(shape, dtype))
    n_params = len(in_names)
    n_outs = len(out_avals)
    all_names = in_names + out_names
    if partition_name is not None:
        all_names.append(partition_name)
    donate = tuple(range(n_params, n_params + n_outs))

    def _body(*args):
        operands = list(args)
        if partition_name is not None:
            operands.append(bass2jax.partition_id_tensor())
        outs = bass2jax._bass_exec_p.bind(
            *operands, out_avals=tuple(out_avals), in_names=tuple(all_names),
            out_names=tuple(out_names), lowering_input_output_aliases=(),
            sim_require_finite=True, sim_require_nnan=True, nc=nc)
        return tuple(outs)

    devices = jax.devices()[:NCORES]
    mesh = Mesh(_np.asarray(devices), ("core",))
    sharded = jax.jit(
        shard_map(_body, mesh=mesh,
                  in_specs=(PartitionSpec("core"),) * (n_params + n_outs),
                  out_specs=(PartitionSpec("core"),) * n_outs,
                  check_rep=False),
        donate_argnums=donate, keep_unused=True)

    concat_in = [
        _np.concatenate([_np.asarray(in_maps[c][n]) for c in range(NCORES)],
                        axis=0)
        for n in in_names]
    dev_in = [jax.device_put(a) for a in concat_in]
    times = []
    for _ in range(repeats):
        zeros = [jax.device_put(
            _np.zeros((NCORES * z.shape[0], *z.shape[1:]), z.dtype))
            for z in zero_outs]
        for z in zeros:
            z.block_until_ready()
        t0 = time.time()
        outs = sharded(*dev_in, *zeros)
        for o in outs:
            o.block_until_ready()
        times.append(time.time() - t0)
    return times

